# revision 8
# baseline (speedup 1.0000x reference)
"""Trainium2 Bass kernel for the ensembler vote-histogram problem.

Computation (reference):
    pred = argmax(expert_logits, axis=-1)            # [E, B, S]
    w    = 1 + noise * 0.001                         # [E, B, S]
    out[b,s,c] = sum_e w[e,b,s] * (pred[e,b,s] == c) # [B, S, C]

Shapes: expert_logits [10, 128, 4000, 5] f32, noise [10, 128, 4000] f32,
out [128, 4000, 5] f32.

Strategy (8 NeuronCores, data-parallel over the batch dim, 16 batches/core):

Per core (64000 tokens, 10 experts, 5 classes):
- SBUF partition layout: 120 partitions = (group g in 0..11) x (expert e in
  0..9), p = 10*g + e.  Each row holds a contiguous run of TAU=5334 tokens
  (group 11 overlaps group 10 by 8 tokens so 12*5334 covers 64000 exactly;
  the overlap is computed twice and stored once).
- VectorE: in-place prefix-max over the 5 classes (4 strided max ops), then
  PM_c = (max <= prefix_max_c) -- a 0/1 step function that rises at the
  FIRST argmax (tie-exact), then nPM_c = 0.001 * noise * PM_c.  PM/nPM are
  written in bf16 into a 6-slot-per-token "gap" layout whose slot 0 is
  always zero.
- TensorE: votes = sum_e (PM_c - PM_{c-1}) + sum_e (nPM_c - nPM_{c-1}),
  computed as 4 accumulating matmuls per PSUM sub-tile with a constant
  block lhs (+1 / -1 per group); the c-1 shift is just the gap view offset
  by one slot, with slot 0 providing the PM_{-1} = 0 column.
- ScalarE copies PSUM -> SBUF; DMA stores contiguous token-major output.
"""
import sys

sys.path.insert(0, "/opt/trn_rl_repo")

import numpy as np

E, B, S, C = 10, 128, 4000, 5
NCORES = 8
BL = B // NCORES            # 16 batches per core
NTOK = BL * S               # 64000 tokens per core
G = 12                      # token groups -> 120 partitions
P = G * E
TAU = 5334                  # tokens per group row
GS = [g * TAU for g in range(G - 1)] + [NTOK - TAU]   # group start tokens
ROW11_SKIP = GS[G - 2] + TAU - GS[G - 1]              # 8 overlap tokens
TC = 1632                   # DVE chunk tokens (multiple of 408)
SBUF_BUFS = 2               # x/n/pm/npm buffering depth
PSUM_BUFS = 2
STAGE_BUFS = 3
SUBT = 102                  # tokens per matmul (510 columns <= 512, 1 bank)
BANKW = 512                 # f32 elements per PSUM bank
PSUB = 4                    # sub-tiles (banks) per PSUM tile -> 408 tokens

_CACHE = {}


VARIANT = "v3"


def _build(variant=None):
    import concourse.bacc as bacc
    import concourse.mybir as mybir
    import concourse.tile as tile

    if variant is None:
        variant = VARIANT
    nc = bacc.Bacc("TRN2", target_bir_lowering=False, debug=False,
                   num_devices=NCORES)
    xd = nc.dram_tensor("expert_logits", (E, BL, S, C), mybir.dt.float32,
                        kind="ExternalInput").ap()
    nd = nc.dram_tensor("noise", (E, BL, S), mybir.dt.float32,
                        kind="ExternalInput").ap()
    od = nc.dram_tensor("out", (BL, S, C), mybir.dt.float32,
                        kind="ExternalOutput").ap()

    with tile.TileContext(nc) as tc:
        if variant.startswith("v3"):
            with tc.tile_pool(name="const", bufs=1) as cpool:
                consts = _consts_v3(tc, cpool)
                _kernel_v3(tc, od, xd, nd, consts, variant=variant)
        elif variant.startswith("v2"):
            _kernel_v2(tc, od, xd, nd, variant=variant)
        else:
            _kernel(tc, od, xd, nd, variant=variant)
    nc.compile()
    return nc


def _consts_v3(tc, cpool):
    """One-time constants: lhsT masks, scan reset bias, class-4 mask."""
    import concourse.mybir as mybir

    nc = tc.nc
    op = mybir.AluOpType
    fp16 = mybir.dt.float16
    f32 = mybir.dt.float32
    KC = 4

    lpA = cpool.tile([128, 16], fp16, tag="lpA")
    lmA = cpool.tile([128, 16], fp16, tag="lmA")
    lpB = [cpool.tile([128, 16], fp16, tag=f"lpB{k}", name=f"lpB{k}")
           for k in range(KC)]
    lmB = [cpool.tile([128, 16], fp16, tag=f"lmB{k}", name=f"lmB{k}")
           for k in range(KC)]
    vio = cpool.tile([128, 16], f32, tag="vio")
    tmp = cpool.tile([128, 16], f32, tag="tmp")
    # v[p, m] = p - m; p mod 16 == m  <=>  v in {0, 16, ..., 112}
    nc.gpsimd.iota(vio[:], pattern=[[-1, 16]], base=0, channel_multiplier=1,
                   allow_small_or_imprecise_dtypes=True)
    nc.vector.tensor_scalar(out=lpA[:], in0=vio[:], scalar1=0.0,
                            scalar2=None, op0=op.is_equal)
    for e in range(1, 8):
        nc.vector.tensor_scalar(out=tmp[:], in0=vio[:], scalar1=float(16 * e),
                                scalar2=None, op0=op.is_equal)
        nc.vector.tensor_tensor(out=lpA[:], in0=lpA[:], in1=tmp[:], op=op.add)
    nc.vector.tensor_scalar(out=lmA[:], in0=lpA[:], scalar1=-1.0,
                            scalar2=None, op0=op.mult)
    for k in range(KC):
        # v[p, m] = p - 4m - k; p mod 64 == 4m+k  <=>  v in {0, 64}
        nc.gpsimd.iota(vio[:], pattern=[[-4, 16]], base=-k,
                       channel_multiplier=1,
                       allow_small_or_imprecise_dtypes=True)
        nc.vector.tensor_scalar(out=lpB[k][:], in0=vio[:], scalar1=0.0,
                                scalar2=None, op0=op.is_equal)
        nc.vector.tensor_scalar(out=tmp[:], in0=vio[:], scalar1=64.0,
                                scalar2=None, op0=op.is_equal)
        nc.vector.tensor_tensor(out=lpB[k][:], in0=lpB[k][:], in1=tmp[:],
                                op=op.add)
        nc.vector.tensor_scalar(out=lmB[k][:], in0=lpB[k][:], scalar1=-1.0,
                                scalar2=None, op0=op.mult)

    # bias5: [-30000, 0, 0, 0, 0] per token -- segmented-scan reset
    bias5 = cpool.tile([128, 5000], fp16, tag="bias5")
    b5v = bias5[:].rearrange("p (t c) -> p t c", c=C)
    nc.gpsimd.memset(bias5[:], 0.0)
    nc.gpsimd.memset(b5v[:, :, 0], -30000.0)
    # mask04: [1, 1, 1, 1, 0] per token -- zeroes class-4 for the shifted rhs
    mask04 = cpool.tile([128, 5000], fp16, tag="mask04")
    m4v = mask04[:].rearrange("p (t c) -> p t c", c=C)
    nc.gpsimd.memset(mask04[:], 1.0)
    nc.gpsimd.memset(m4v[:, :, 4], 0.0)
    return dict(lpA=lpA, lmA=lmA, lpB=lpB, lmB=lmB, bias5=bias5,
                mask04=mask04)


def _kernel_v3(tc, od, xd, nd, consts, variant="v3"):
    """Scan-based 128-partition kernel: all elementwise ops contiguous.

    Per 1000-seq chunk (and once for the B experts):
      DVE  : Q = segmented prefix-max scan (reset via bias5), in place on x;
             wpmF = (Q4 <= Q_c) in fp16 (token-major, contiguous).
      Pool : wpmF *= w;  wpm5 = wpmF * mask04 (class-4 zeroed, written at
             +1 column into a lead-zero tile for the shifted rhs).
      DVE  : w = 1 + 0.001 * noise.
      PE   : psum[b, t, c] = sum_p lpX[p,b] * wpmF[p, 5t+c]
                           - sum_p lpX[p,b] * wpm5[p, 5t+c(shifted)]
      Act  : PSUM -> SBUF staging; one store DMA per chunk (gpsimd).
    """
    import concourse.mybir as mybir

    nc = tc.nc
    op = mybir.AluOpType
    fp16 = mybir.dt.float16
    f32 = mybir.dt.float32

    xf = xd.rearrange("e b s c -> (e b s c)")
    nf = nd.rearrange("e b s -> (e b s)")
    xA = xf[:8 * BL * S * C].rearrange("(p w) -> p w", p=128)   # [128, 20000]
    xB = xf[8 * BL * S * C:].rearrange("(p w) -> p w", p=128)   # [128, 5000]
    nA = nf[:8 * BL * S].rearrange("(p w) -> p w", p=128)       # [128, 4000]
    nB = nf[8 * BL * S:].rearrange("(p w) -> p w", p=128)       # [128, 1000]
    og = od.rearrange("b s c -> b (s c)")                       # [16, 20000]

    KC, TCH = 4, 1000
    SUBT = 100
    PSUB = 2
    lpA, lmA = consts["lpA"], consts["lmA"]
    lpB, lmB = consts["lpB"], consts["lmB"]
    bias5, mask04 = consts["bias5"], consts["mask04"]

    with tc.tile_pool(name="persist", bufs=1) as bpool, \
         tc.tile_pool(name="stream", bufs=2) as pool, \
         tc.tile_pool(name="psum", bufs=4, space="PSUM") as ppool, \
         tc.tile_pool(name="stage", bufs=2) as spool:

        def prep(xt, ntile, wt, wpmF, wpm5):
            """x -> wpmF/wpm5 fp16 tiles (5000 cols, wpm5 has lead zero)."""
            # segmented prefix-max along classes, in place
            nc.vector.tensor_tensor_scan(
                out=xt[:], data0=bias5[:], data1=xt[:], initial=0.0,
                op0=op.add, op1=op.max)
            xv = xt[:].rearrange("p (t c) -> p t c", c=C)
            m_b = xv[:, :, C - 1:C].broadcast_to((128, TCH, C))
            wv = wpmF[:].rearrange("p (t c) -> p t c", c=C)
            nc.vector.tensor_tensor(out=wv[:, :, :], in0=m_b,
                                    in1=xv[:, :, :], op=op.is_le)
            nc.vector.tensor_scalar(out=wt[:], in0=ntile[:], scalar1=0.001,
                                    scalar2=1.0, op0=op.mult, op1=op.add)
            w_b = wt[:].unsqueeze(2).broadcast_to((128, TCH, C))
            nc.gpsimd.tensor_tensor(out=wv[:, :, :], in0=wv[:, :, :],
                                    in1=w_b, op=op.mult)
            nc.gpsimd.tensor_tensor(out=wpm5[:, 1:5001], in0=wpmF[:],
                                    in1=mask04[:], op=op.mult)

        # ---- B (experts 8, 9): load + process once
        xbt = bpool.tile([128, 5000], f32, tag="xB")
        nbt = bpool.tile([128, 1000], f32, tag="nB")
        wbt = bpool.tile([128, 1000], f32, tag="wB")
        wpmFB = bpool.tile([128, 5000], fp16, tag="wpmFB")
        wpm5B = bpool.tile([128, 5008], fp16, tag="wpm5B")
        nc.sync.dma_start(out=xbt[:], in_=xB[:, :])
        nc.scalar.dma_start(out=nbt[:], in_=nB[:, :])
        nc.vector.memset(wpm5B[:, 0:1], 0.0)
        if variant != "v3dma":
            prep(xbt, nbt, wbt, wpmFB, wpm5B)

        # ---- stream A chunks
        for k in range(KC):
            xt = pool.tile([128, 5000], f32, tag="x")
            ntile = pool.tile([128, 1000], f32, tag="n")
            wt = pool.tile([128, 1000], f32, tag="w")
            wpmF = pool.tile([128, 5000], fp16, tag="wpmF")
            wpm5 = pool.tile([128, 5008], fp16, tag="wpm5")
            nc.sync.dma_start(out=xt[:], in_=xA[:, k * 5000:(k + 1) * 5000])
            nc.scalar.dma_start(out=ntile[:],
                                in_=nA[:, k * 1000:(k + 1) * 1000])
            if variant == "v3dma":
                continue
            nc.vector.memset(wpm5[:, 0:1], 0.0)
            prep(xt, ntile, wt, wpmF, wpm5)
            if variant == "v3prep":
                continue

            st = spool.tile([16, 5000], f32, tag="st")
            t0 = 0
            while t0 < TCH:
                ntok = min(PSUB * SUBT, TCH - t0)
                nbank = -(-ntok // SUBT)
                ps = ppool.tile([16, PSUB * 512], f32, tag="ps")
                specs = ((lpA, wpmF, 0, True, False),
                         (lmA, wpm5, 0, False, False),
                         (lpB[k], wpmFB, 0, False, False),
                         (lmB[k], wpm5B, 0, False, True))
                for lhs, src, _, first, last in specs:
                    for j in range(nbank):
                        tj = t0 + j * SUBT
                        tl = min(SUBT, TCH - tj)
                        nc.tensor.matmul(
                            out=ps[:, j * 512:j * 512 + tl * C],
                            lhsT=lhs[:], rhs=src[:, tj * C:(tj + tl) * C],
                            start=first, stop=last, skip_group_check=True)
                for j in range(nbank):
                    tj = t0 + j * SUBT
                    tl = min(SUBT, TCH - tj)
                    nc.scalar.copy(out=st[:, tj * C:tj * C + tl * C],
                                   in_=ps[:, j * 512:j * 512 + tl * C])
                t0 += ntok
            nc.gpsimd.dma_start(out=og[:, k * 5000:(k + 1) * 5000],
                                in_=st[:, :])


def _kernel_v2(tc, od, xd, nd, variant="v2"):
    """128-partition flat-load kernel.

    Loads use only [128, w] single-level access patterns (measured ~2.4x
    faster than the 110/120-partition 2-level APs of v1: ~350 vs ~150
    GB/s/core).

    Layout A (experts 0..7): x flat -> [128, 20000]; partition p = 16e+b
    holds batch b of expert e, all 4000 seq positions (20000 f32).
    Layout B (experts 8..9): x tail -> [128, 5000]; partition
    p = 64(e-8) + 4b + q holds seq quarter q (1000 positions) of batch b.

    Per seq-chunk of 1000 positions (4 chunks):
      DVE: in-place prefix-max over the 5 classes (4 strided max ops),
      then PM_c = (max <= prefix_c) for c=0..3 into fp16 "gap" slots 1..4
      (slot 0 stays zero, slot 5 = w = 1+0.001*noise).
      Pool: slots 1..4 *= w.  Act: slot5 copy + PSUM->SBUF drains.
      PE: votes[b, t, c] = sum_p lhs[p, b] * (wpm[p,t,1+c] - wpm[p,t,c])
      as 4 accumulating matmul passes (lpA/lmA over A, lpB_k/lmB_k over
      B with the chunk-index-k partition mask).
    Output groups = batches: out [16, 20000], one store DMA per chunk.
    """
    import concourse.mybir as mybir

    nc = tc.nc
    op = mybir.AluOpType
    fp16 = mybir.dt.float16
    f32 = mybir.dt.float32

    xf = xd.rearrange("e b s c -> (e b s c)")
    nf = nd.rearrange("e b s -> (e b s)")
    xA = xf[:8 * BL * S * C].rearrange("(p w) -> p w", p=128)   # [128, 20000]
    xB = xf[8 * BL * S * C:].rearrange("(p w) -> p w", p=128)   # [128, 5000]
    nA = nf[:8 * BL * S].rearrange("(p w) -> p w", p=128)       # [128, 4000]
    nB = nf[8 * BL * S:].rearrange("(p w) -> p w", p=128)       # [128, 1000]
    og = od.rearrange("b s c -> b (s c)")                       # [16, 20000]

    KC, TCH = 4, 1000       # seq chunks
    SUBT = 100              # tokens per PSUM bank (500 of 512 cols)
    PSUB = 2                # banks per PSUM tile

    with tc.tile_pool(name="const", bufs=1) as cpool, \
         tc.tile_pool(name="persist", bufs=1) as bpool, \
         tc.tile_pool(name="stream", bufs=SBUF_BUFS) as pool, \
         tc.tile_pool(name="psum", bufs=4, space="PSUM") as ppool, \
         tc.tile_pool(name="stage", bufs=STAGE_BUFS) as spool:
        # ---- constant lhsT masks [128, 16] fp16
        # lpA[p, m] = (p mod 16 == m); lpB_k[p, m] = (p mod 64 == 4m + k)
        lpA = cpool.tile([128, 16], fp16, tag="lpA")
        lmA = cpool.tile([128, 16], fp16, tag="lmA")
        lpB = [cpool.tile([128, 16], fp16, tag=f"lpB{k}", name=f"lpB{k}")
               for k in range(KC)]
        lmB = [cpool.tile([128, 16], fp16, tag=f"lmB{k}", name=f"lmB{k}")
               for k in range(KC)]
        vio = cpool.tile([128, 16], f32, tag="vio")
        tmp = cpool.tile([128, 16], f32, tag="tmp")
        # v[p, m] = p - m; p mod 16 == m  <=>  v in {0, 16, ..., 112}
        nc.gpsimd.iota(vio[:], pattern=[[-1, 16]], base=0,
                       channel_multiplier=1,
                       allow_small_or_imprecise_dtypes=True)
        nc.vector.tensor_scalar(out=lpA[:], in0=vio[:], scalar1=0.0,
                                scalar2=None, op0=op.is_equal)
        for e in range(1, 8):
            nc.vector.tensor_scalar(out=tmp[:], in0=vio[:],
                                    scalar1=float(16 * e),
                                    scalar2=None, op0=op.is_equal)
            nc.vector.tensor_tensor(out=lpA[:], in0=lpA[:], in1=tmp[:],
                                    op=op.add)
        nc.vector.tensor_scalar(out=lmA[:], in0=lpA[:], scalar1=-1.0,
                                scalar2=None, op0=op.mult)
        for k in range(KC):
            # v[p, m] = p - 4m - k; p mod 64 == 4m+k  <=>  v in {0, 64}
            nc.gpsimd.iota(vio[:], pattern=[[-4, 16]], base=-k,
                           channel_multiplier=1,
                           allow_small_or_imprecise_dtypes=True)
            nc.vector.tensor_scalar(out=lpB[k][:], in0=vio[:], scalar1=0.0,
                                    scalar2=None, op0=op.is_equal)
            nc.vector.tensor_scalar(out=tmp[:], in0=vio[:], scalar1=64.0,
                                    scalar2=None, op0=op.is_equal)
            nc.vector.tensor_tensor(out=lpB[k][:], in0=lpB[k][:],
                                    in1=tmp[:], op=op.add)
            nc.vector.tensor_scalar(out=lmB[k][:], in0=lpB[k][:],
                                    scalar1=-1.0, scalar2=None, op0=op.mult)

        def prep(xt, ntile, wt, pmg, n_tok):
            """prefix-max + PM + weights into the gap tile (slots 1..5)."""
            xv = xt[:].rearrange("p (t c) -> p t c", c=C)
            for c in range(1, C):
                nc.vector.tensor_max(out=xv[:, :, c], in0=xv[:, :, c],
                                     in1=xv[:, :, c - 1])
            m_b = xv[:, :, C - 1:C].broadcast_to((128, n_tok, C - 1))
            nc.vector.tensor_tensor(out=pmg[:, :, 1:C], in0=m_b,
                                    in1=xv[:, :, 0:C - 1], op=op.is_le)
            nc.vector.tensor_scalar(out=wt[:], in0=ntile[:], scalar1=0.001,
                                    scalar2=1.0, op0=op.mult, op1=op.add)
            nc.scalar.copy(out=pmg[:, :, C], in_=wt[:])
            w_b = wt[:].unsqueeze(2).broadcast_to((128, n_tok, C - 1))
            nc.gpsimd.tensor_tensor(out=pmg[:, :, 1:C], in0=pmg[:, :, 1:C],
                                    in1=w_b, op=op.mult)

        # ---- B (experts 8, 9): load + process once
        xbt = bpool.tile([128, 5000], f32, tag="xB")
        nbt = bpool.tile([128, 1000], f32, tag="nB")
        wbt = bpool.tile([128, 1000], f32, tag="wB")
        pmB = bpool.tile([128, 6000], fp16, tag="pmB")
        nc.sync.dma_start(out=xbt[:], in_=xB[:, :])
        nc.scalar.dma_start(out=nbt[:], in_=nB[:, :])
        pmBg = pmB[:].rearrange("p (t s) -> p t s", s=C + 1)
        if variant != "v2dma":
            nc.gpsimd.memset(pmBg[:, :, 0], 0.0)
            prep(xbt, nbt, wbt, pmBg, 1000)

        # ---- stream A chunks
        for k in range(KC):
            xt = pool.tile([128, 5000], f32, tag="x")
            ntile = pool.tile([128, 1000], f32, tag="n")
            wt = pool.tile([128, 1000], f32, tag="w")
            pmA = pool.tile([128, 6000], fp16, tag="pmA")
            nc.sync.dma_start(out=xt[:], in_=xA[:, k * 5000:(k + 1) * 5000])
            nc.scalar.dma_start(out=ntile[:],
                                in_=nA[:, k * 1000:(k + 1) * 1000])
            if variant == "v2dma":
                continue
            pmAg = pmA[:].rearrange("p (t s) -> p t s", s=C + 1)
            nc.gpsimd.memset(pmAg[:, :, 0], 0.0)
            prep(xt, ntile, wt, pmAg, 1000)
            if variant == "v2prep":
                continue

            st = spool.tile([16, 5000], f32, tag="st")
            t0 = 0
            while t0 < TCH:
                ntok = min(PSUB * SUBT, TCH - t0)
                nbank = -(-ntok // SUBT)
                ps = ppool.tile([16, PSUB * 512], f32, tag="ps")
                specs = ((lpA, pmAg, 1, True, False),
                         (lmA, pmAg, 0, False, False),
                         (lpB[k], pmBg, 1, False, False),
                         (lmB[k], pmBg, 0, False, True))
                for lhs, src, ofs, first, last in specs:
                    for j in range(nbank):
                        tj = t0 + j * SUBT
                        tl = min(SUBT, TCH - tj)
                        nc.tensor.matmul(
                            out=ps[:, j * 512:j * 512 + tl * C],
                            lhsT=lhs[:], rhs=src[:, tj:tj + tl, ofs:ofs + C],
                            start=first, stop=last, skip_group_check=True)
                for j in range(nbank):
                    tj = t0 + j * SUBT
                    tl = min(SUBT, TCH - tj)
                    nc.scalar.copy(out=st[:, tj * C:tj * C + tl * C],
                                   in_=ps[:, j * 512:j * 512 + tl * C])
                t0 += ntok
            nc.gpsimd.dma_start(out=og[:, k * 5000:(k + 1) * 5000],
                                in_=st[:, :])


def _kernel(tc, od, xd, nd, variant="full"):
    import concourse.mybir as mybir

    nc = tc.nc
    op = mybir.AluOpType
    xf = xd.rearrange("e b s c -> e (b s c)")            # [10, 320000]
    nf = nd.rearrange("e b s -> e (b s)")                # [10, 64000]
    of = od.rearrange("b s c -> (b s c)").unsqueeze(0)   # [1, 320000]

    # DRAM views for the uniform groups 0..10 (partition-majorised (g e))
    xa = xf[:, :(G - 1) * TAU * C].rearrange("e (g w) -> e g w", g=G - 1) \
        .transpose([1, 0, 2])                            # [11, 10, TAU*C]
    na = nf[:, :(G - 1) * TAU].rearrange("e (g w) -> e g w", g=G - 1) \
        .transpose([1, 0, 2])                            # [11, 10, TAU]
    # group 11 rows
    xb = xf[:, GS[G - 1] * C:GS[G - 1] * C + TAU * C]
    nb = nf[:, GS[G - 1]:GS[G - 1] + TAU]
    # output rows for groups 0..10
    oa = of[:, :(G - 1) * TAU * C].rearrange("o (g w) -> (o g) w", g=G - 1)

    with tc.tile_pool(name="const", bufs=1) as cpool, \
         tc.tile_pool(name="sbuf", bufs=SBUF_BUFS) as pool, \
         tc.tile_pool(name="psum", bufs=PSUM_BUFS, space="PSUM") as ppool, \
         tc.tile_pool(name="stage", bufs=STAGE_BUFS) as spool:
        # Constant lhsT tiles: lp[p, m] = (p // 10 == m), lm = -lp,
        # built from iota v[p, m] = 10*m - p   (p//10 == m  <=>  -9 <= v <= 0)
        lp = cpool.tile([P, G], mybir.dt.bfloat16)
        lm = cpool.tile([P, G], mybir.dt.bfloat16)
        vio = cpool.tile([P, G], mybir.dt.float32)
        va = cpool.tile([P, G], mybir.dt.float32)
        nc.gpsimd.iota(vio[:], pattern=[[10, G]], base=0, channel_multiplier=-1,
                       allow_small_or_imprecise_dtypes=True)
        nc.vector.tensor_scalar(out=va[:], in0=vio[:], scalar1=-9.0,
                                scalar2=None, op0=op.is_ge)
        nc.vector.scalar_tensor_tensor(out=lp[:], in0=vio[:], scalar=0.0,
                                       in1=va[:], op0=op.is_le, op1=op.mult)
        nc.vector.tensor_scalar(out=lm[:], in0=lp[:], scalar1=-1.0,
                                scalar2=None, op0=op.mult)

        for t0 in range(0, TAU, TC):
            tcl = min(TC, TAU - t0)
            xt = pool.tile([P, TC * C], mybir.dt.float32, tag="x")
            nt = pool.tile([P, TC], mybir.dt.float32, tag="n")
            pm = pool.tile([P, TC * (C + 1)], mybir.dt.bfloat16, tag="pm")
            npm = pool.tile([P, TC * (C + 1)], mybir.dt.bfloat16, tag="npm")

            # loads: splitld variants load x in piece-aligned sub-loads so
            # the first DVE piece starts after ~1MB instead of ~4MB
            if variant == "splitld":
                lo = 0
                while lo < tcl:
                    hi = min(lo + PSUB * SUBT, tcl)
                    nc.sync.dma_start(
                        out=xt[:(G - 1) * E, lo * C:hi * C],
                        in_=xa[:, :, (t0 + lo) * C:(t0 + hi) * C])
                    nc.sync.dma_start(
                        out=xt[(G - 1) * E:P, lo * C:hi * C],
                        in_=xb[:, (t0 + lo) * C:(t0 + hi) * C])
                    lo = hi
            else:
                nc.sync.dma_start(out=xt[:(G - 1) * E, :tcl * C],
                                  in_=xa[:, :, t0 * C:(t0 + tcl) * C])
                nc.sync.dma_start(out=xt[(G - 1) * E:P, :tcl * C],
                                  in_=xb[:, t0 * C:(t0 + tcl) * C])
            ndma = nc.gpsimd if variant == "gpall" else nc.sync
            ndma.dma_start(out=nt[:(G - 1) * E, :tcl],
                           in_=na[:, :, t0:t0 + tcl])
            ndma.dma_start(out=nt[(G - 1) * E:P, :tcl],
                           in_=nb[:, t0:t0 + tcl])

            if variant == "dmaonly":
                continue

            # views over the whole chunk
            xv = xt[:, :tcl * C].rearrange("p (t c) -> p t c", c=C)
            pmg = pm[:, :tcl * (C + 1)].rearrange("p (t s) -> p t s", s=C + 1)
            npmg = npm[:, :tcl * (C + 1)].rearrange("p (t s) -> p t s", s=C + 1)
            nc.gpsimd.memset(pmg[:, :, 0], 0.0)
            nc.gpsimd.memset(npmg[:, :, 0], 0.0)

            def dve_piece(lo, hi):
                # in-place prefix max along classes: x becomes Q
                for c in range(1, C):
                    nc.vector.tensor_max(out=xv[:, lo:hi, c],
                                         in0=xv[:, lo:hi, c],
                                         in1=xv[:, lo:hi, c - 1])
                m_b = xv[:, lo:hi, C - 1:C].broadcast_to((P, hi - lo, C))
                nc.vector.tensor_tensor(out=pmg[:, lo:hi, 1:C + 1], in0=m_b,
                                        in1=xv[:, lo:hi, :], op=op.is_le)
                n_b = nt[:, lo:hi].unsqueeze(2).broadcast_to((P, hi - lo, C))
                if variant != "nonpm":
                    nc.vector.scalar_tensor_tensor(
                        out=npmg[:, lo:hi, 1:C + 1],
                        in0=pmg[:, lo:hi, 1:C + 1], scalar=0.001, in1=n_b,
                        op0=op.mult, op1=op.mult)

            def dve_pm_piece(lo, hi):
                m_b = xv[:, lo:hi, C - 1:C].broadcast_to((P, hi - lo, C))
                nc.vector.tensor_tensor(out=pmg[:, lo:hi, 1:C + 1], in0=m_b,
                                        in1=xv[:, lo:hi, :], op=op.is_le)
                n_b = nt[:, lo:hi].unsqueeze(2).broadcast_to((P, hi - lo, C))
                nc.vector.scalar_tensor_tensor(
                    out=npmg[:, lo:hi, 1:C + 1],
                    in0=pmg[:, lo:hi, 1:C + 1], scalar=0.001, in1=n_b,
                    op0=op.mult, op1=op.mult)

            if variant == "finemix":
                for c in range(1, C):
                    nc.vector.tensor_max(out=xv[:, :, c], in0=xv[:, :, c],
                                         in1=xv[:, :, c - 1])
            elif variant not in ("fine", "actdma", "gpdma", "splitld", "gpall"):
                dve_piece(0, tcl)

            # PE + PSUM->SBUF + store, per PSUM tile of up to PSUB banks
            tt0 = 0
            while tt0 < tcl:
                nsub = min(PSUB, -(-(tcl - tt0) // SUBT))
                tok_here = min(PSUB * SUBT, tcl - tt0)
                if variant in ("fine", "actdma", "gpdma", "splitld", "gpall"):
                    dve_piece(tt0, tt0 + tok_here)
                elif variant == "finemix":
                    dve_pm_piece(tt0, tt0 + tok_here)
                ps = ppool.tile([G, PSUB * BANKW], mybir.dt.float32, tag="ps")
                subs = []
                for j in range(nsub):
                    tj = tt0 + j * SUBT
                    tl = min(SUBT, tcl - tj)
                    subs.append((j, tj, tl))
                # order matmuls to minimise weight swaps: lp streams then lm
                for sign, lhs, ofs in ((0, lp, 1), (1, lm, 0)):
                    for k, (j, tj, tl) in enumerate(subs):
                        full = pmg[:, tj:tj + tl, ofs:ofs + C]
                        nfull = npmg[:, tj:tj + tl, ofs:ofs + C]
                        is_first = (sign == 0)
                        is_last = (sign == 1)
                        if variant == "nonpm":
                            nc.tensor.matmul(
                                out=ps[:, j * BANKW:j * BANKW + tl * C],
                                lhsT=lhs[:], rhs=full,
                                start=is_first, stop=is_last,
                                skip_group_check=True)
                        else:
                            nc.tensor.matmul(
                                out=ps[:, j * BANKW:j * BANKW + tl * C],
                                lhsT=lhs[:], rhs=full,
                                start=is_first, stop=False,
                                skip_group_check=True)
                            nc.tensor.matmul(
                                out=ps[:, j * BANKW:j * BANKW + tl * C],
                                lhsT=lhs[:], rhs=nfull,
                                start=False, stop=is_last,
                                skip_group_check=True)

                # PSUM -> SBUF (ScalarE), one op per PSUM tile
                st = spool.tile([G, PSUB * SUBT * C], mybir.dt.float32,
                                tag="st")
                for j, tj, tl in subs:
                    nc.scalar.copy(out=st[:, j * SUBT * C:j * SUBT * C + tl * C],
                                   in_=ps[:, j * BANKW:j * BANKW + tl * C])

                # stores: groups 0..10 in one DMA; group 11 separately
                glo = t0 + tt0                     # group-local token start
                ghi = glo + tok_here
                sdma = {"actdma": nc.scalar, "gpdma": nc.gpsimd, "splitld": nc.gpsimd, "gpall": nc.gpsimd}.get(variant, nc.sync)
                sdma.dma_start(out=oa[:, glo * C:ghi * C],
                               in_=st[:G - 1, :tok_here * C])
                if glo >= ROW11_SKIP:
                    sdma.dma_start(
                        out=of[:, (GS[G - 1] + glo) * C:(GS[G - 1] + ghi) * C],
                        in_=st[G - 1:G, :tok_here * C])
                else:
                    skip = ROW11_SKIP - glo
                    sdma.dma_start(
                        out=of[:, (GS[G - 1] + ROW11_SKIP) * C:
                               (GS[G - 1] + ghi) * C],
                        in_=st[G - 1:G, skip * C:tok_here * C])
                tt0 += tok_here


def _build_loop(reps, variant="full"):
    """Benchmark variant: the whole kernel body repeated `reps` times inside
    an on-device For_i loop, so device time dominates host dispatch."""
    import concourse.bacc as bacc
    import concourse.mybir as mybir
    import concourse.tile as tile

    nc = bacc.Bacc("TRN2", target_bir_lowering=False, debug=False,
                   num_devices=NCORES)
    xd = nc.dram_tensor("expert_logits", (E, BL, S, C), mybir.dt.float32,
                        kind="ExternalInput").ap()
    nd = nc.dram_tensor("noise", (E, BL, S), mybir.dt.float32,
                        kind="ExternalInput").ap()
    od = nc.dram_tensor("out", (BL, S, C), mybir.dt.float32,
                        kind="ExternalOutput").ap()
    with tile.TileContext(nc) as tc:
        if variant.startswith("v3"):
            with tc.tile_pool(name="const", bufs=1) as cpool:
                consts = _consts_v3(tc, cpool)
                with tc.For_i(0, reps, 1,
                              hint_engines=(mybir.EngineType.PE,
                                            mybir.EngineType.SP)):
                    _kernel_v3(tc, od, xd, nd, consts, variant=variant)
        else:
            with tc.For_i(0, reps, 1,
                          hint_engines=(mybir.EngineType.PE,
                                        mybir.EngineType.SP)):
                if variant.startswith("v2"):
                    _kernel_v2(tc, od, xd, nd, variant=variant)
                else:
                    _kernel(tc, od, xd, nd, variant=variant)
    nc.compile()
    return nc


def _get_nc():
    if "nc" not in _CACHE:
        _CACHE["nc"] = _build()
    return _CACHE["nc"]


def _run(inputs, trace=False):
    from concourse import bass_utils

    nc = _get_nc()
    x = np.ascontiguousarray(inputs["expert_logits"], dtype=np.float32)
    n = np.ascontiguousarray(inputs["noise"], dtype=np.float32)
    in_maps = []
    for k in range(NCORES):
        bsl = slice(k * BL, (k + 1) * BL)
        in_maps.append({
            "expert_logits": np.ascontiguousarray(x[:, bsl]),
            "noise": np.ascontiguousarray(n[:, bsl]),
        })
    res = bass_utils.run_bass_kernel_spmd(
        nc, in_maps, core_ids=list(range(NCORES)), trace=trace)
    out = np.concatenate([r["out"] for r in res.results], axis=0)
    return out, res


def kernel(**inputs) -> np.ndarray:
    out, _ = _run(inputs, trace=False)
    return out

